# revision 10
# baseline (speedup 1.0000x reference)
"""AutoCorrelation Trainium2 kernel.

Reference reformulation (verified to 3e-7 rel):
  H=8, L=2048, D=512, k_sel=4, SCALE=1/(H*L)
  qbar = sum_l queries[b,l,:];  qs = qbar @ wq;  t = wk @ qs
  mean_corr = (keys[b] @ t) * SCALE                     # [2048]
  top_idx, top_vals = top_k(mean_corr, 4); w = softmax(top_vals)
  Vp = values[b] @ wv                                   # [2048, 512]
  Aw = sum_j w_j * roll(Vp, -top_idx_j, axis=0)         # [2048, 512]
  # reference's transpose(0,3,1,2).reshape quirk => per output row i:
  #   r = i%4, c = ((i%32)//4)*64 + i//32
  #   out[b,i,:] = Aw[r*512:(r+1)*512, c] @ wo
  => for each r: out_rows(r) = Aw[r*512:(r+1)*512, :].T @ wo

Sharding: 8 cores = 4 batches x 2 channel-halves (d half of each head).
Each core redundantly computes the tiny front-end (top-k weights) for its
batch and produces the 1024 output rows whose channels fall in its half.

Device program per core:
  - qbar via PE (ones^T @ q-tiles, PSUM accumulate), qs via PE
  - t = rowsum(wk * bcast(qs)), mc = rowsum(keys * bcast(t)) on DVE
  - top-8 via DVE max/max_index on mc flattened to [1,2048] (DRAM bounce)
  - softmax on top-4 (ACT exp), w_j * I identities on DVE
  - VpT[c_half, l] = wv_half^T @ values^T  (PE, from host-transposed vt)
  - AwT[c, l-block] accumulated over j in PSUM via scaled-identity matmuls
    with register-offset slices (roll == dynamic free-dim slice on doubled VpT)
  - PE-transpose AwT -> Aw tiles, final GEMM Aw_r^T-tiles @ wo
  All heavy matmuls in float32r (full PE rate, ~1e-7 rel precision).
"""

import numpy as np

B, L, D = 4, 2048, 512
H = 8
K_SEL = 4
SCALE = 1.0 / (H * L)
N_CORES = 8
P = 128
CH = 256          # channels per core (half of 512)
NT = L // P       # 16 l-tiles
DK = D // P       # 4 d-tiles


def _build_nc():
    import concourse.bass as bass
    import concourse.bacc as bacc
    import concourse.mybir as mybir
    from concourse.tile import TileContext
    from concourse.masks import make_identity

    fp32 = mybir.dt.float32
    f32r = mybir.dt.float32r
    u32 = mybir.dt.uint32
    i32 = mybir.dt.int32
    AX = mybir.AxisListType.X
    MUL = mybir.AluOpType.mult

    nc = bacc.Bacc("TRN2", target_bir_lowering=False, debug=False, num_devices=N_CORES)

    q_dram = nc.dram_tensor("q", [L, D], f32r, kind="ExternalInput")
    k_dram = nc.dram_tensor("k", [L, D], fp32, kind="ExternalInput")
    vt_dram = nc.dram_tensor("vt", [D, L], f32r, kind="ExternalInput")  # values^T
    wq_dram = nc.dram_tensor("wq", [D, D], f32r, kind="ExternalInput")
    wk_dram = nc.dram_tensor("wk", [D, D], fp32, kind="ExternalInput")
    wvh_dram = nc.dram_tensor("wvh", [D, CH], f32r, kind="ExternalInput")
    wo_dram = nc.dram_tensor("wo", [D, D], f32r, kind="ExternalInput")
    ones_dram = nc.dram_tensor("ones", [P, 1], f32r, kind="ExternalInput")
    onesr_dram = nc.dram_tensor("onesr", [1, P], fp32, kind="ExternalInput")
    out_dram = nc.dram_tensor("out", [L // 2, D], fp32, kind="ExternalOutput")

    with TileContext(nc) as tc:
        with (
            tc.tile_pool(name="const", bufs=1) as cpool,
            tc.tile_pool(name="wts", bufs=1) as wts,
            tc.tile_pool(name="big", bufs=1) as big,
            tc.tile_pool(name="stream", bufs=2) as stream,
            tc.tile_pool(name="small", bufs=1) as small,
            tc.tile_pool(name="dram", bufs=1, space="DRAM") as dpool,
            tc.tile_pool(name="ps_fe", bufs=2, space="PSUM") as ps_fe,
            tc.tile_pool(name="ps_mm", bufs=3, space="PSUM") as ps_mm,
            tc.tile_pool(name="ps_tp", bufs=2, space="PSUM") as ps_tp,
        ):
            ident = cpool.tile([P, P], fp32, tag="ident")
            make_identity(nc, ident)
            ones_col = cpool.tile([P, 1], f32r, tag="ones")
            nc.sync.dma_start(ones_col, ones_dram[:, :])
            ones_row = cpool.tile([1, P], fp32, tag="onesr")
            nc.sync.dma_start(ones_row, onesr_dram[:, :])

            # ---- weights in ----
            wq_sb = [wts.tile([P, D], f32r, tag=f"wq{i}", name=f"wq{i}") for i in range(DK)]
            wk_sb = [wts.tile([P, D], fp32, tag=f"wk{i}", name=f"wk{i}") for i in range(DK)]
            wvh_sb = [wts.tile([P, CH], f32r, tag=f"wvh{i}", name=f"wvh{i}") for i in range(DK)]
            wo_sb = [wts.tile([P, D], f32r, tag=f"wo{i}", name=f"wo{i}") for i in range(DK)]
            for i in range(DK):
                sl = slice(i * P, (i + 1) * P)
                nc.sync.dma_start(wq_sb[i], wq_dram[sl, :])
                nc.sync.dma_start(wk_sb[i], wk_dram[sl, :])
                nc.sync.dma_start(wvh_sb[i], wvh_dram[sl, :])
                nc.sync.dma_start(wo_sb[i], wo_dram[sl, :])

            # ---- values^T in ----
            vt_sb = [big.tile([P, L], f32r, tag=f"vt{i}", name=f"vt{i}") for i in range(DK)]
            for i in range(DK):
                nc.sync.dma_start(vt_sb[i], vt_dram[i * P:(i + 1) * P, :])

            # ---- VpT = wv_half^T @ values^T, doubled along l, both c-tiles
            # packed in one buffer: cols [ct*4096 + (l % 2048 doubled)] ----
            vpT = big.tile([P, 2 * 2 * L], fp32, tag="vpT", name="vpT")
            for ct in range(2):
                for lc in range(4):  # 512-wide l chunks
                    pv = ps_mm.tile([P, 512], fp32, tag="mm")
                    for dk in range(DK):
                        nc.tensor.matmul(
                            pv,
                            wvh_sb[dk][:, ct * P:(ct + 1) * P],
                            vt_sb[dk][:, lc * 512:(lc + 1) * 512],
                            start=(dk == 0), stop=(dk == DK - 1),
                        )
                    o = ct * 2 * L + lc * 512
                    nc.scalar.copy(vpT[:, o:o + 512], pv)
                # wrap-around doubling
                nc.scalar.copy(
                    vpT[:, ct * 2 * L + L: ct * 2 * L + 2 * L],
                    vpT[:, ct * 2 * L: ct * 2 * L + L])

            # ---- qbar = sum_l queries[l,:] ----
            ps_qbar = ps_fe.tile([1, D], fp32, tag="fe")
            for n in range(NT):
                qt = stream.tile([P, D], f32r, tag="qtile")
                nc.sync.dma_start(qt, q_dram[n * P:(n + 1) * P, :])
                nc.tensor.matmul(
                    ps_qbar, ones_col, qt,
                    start=(n == 0), stop=(n == NT - 1),
                )
            qbar_row = small.tile([1, D], f32r, tag="qbar_row")
            nc.scalar.copy(qbar_row, ps_qbar)

            # qbar^T chunks [128, 4] via DRAM bounce
            d_qb = dpool.tile([1, D], f32r, tag="d_qb")
            nc.sync.dma_start(d_qb, qbar_row)
            qbarT = small.tile([P, DK], f32r, tag="qbarT")
            nc.sync.dma_start(qbarT, d_qb.rearrange("o (c p) -> (o p) c", p=P))

            # qs = qbar @ wq -> [1, 512]
            ps_qs = ps_fe.tile([1, D], fp32, tag="fe")
            for kk in range(DK):
                nc.tensor.matmul(
                    ps_qs, qbarT[:, kk:kk + 1], wq_sb[kk],
                    start=(kk == 0), stop=(kk == DK - 1),
                )
            qs_row = small.tile([1, D], fp32, tag="qs_row")
            nc.scalar.copy(qs_row, ps_qs)

            # broadcast qs to [128, 512] via PE
            ps_qsb = ps_fe.tile([P, D], fp32, tag="fe")
            nc.tensor.matmul(ps_qsb, ones_row, qs_row, start=True, stop=True)
            qs_b = small.tile([P, D], fp32, tag="qs_b")
            nc.scalar.copy(qs_b, ps_qsb)

            # t[i] = sum_m wk[i,m] qs[m]  (DVE)
            tcol = small.tile([P, DK], fp32, tag="tcol")
            for c in range(DK):
                tmp = stream.tile([P, D], fp32, tag="ttmp")
                nc.vector.tensor_mul(tmp, wk_sb[c], qs_b)
                nc.vector.reduce_sum(tcol[:, c:c + 1], tmp, axis=AX)
            d_t = dpool.tile([1, D], fp32, tag="d_t")
            nc.sync.dma_start(d_t.rearrange("o (c p) -> (o p) c", p=P), tcol)
            t_row = small.tile([1, D], fp32, tag="t_row")
            nc.sync.dma_start(t_row, d_t)
            ps_tb = ps_fe.tile([P, D], fp32, tag="fe")
            nc.tensor.matmul(ps_tb, ones_row, t_row, start=True, stop=True)
            t_b = small.tile([P, D], fp32, tag="t_b")
            nc.scalar.copy(t_b, ps_tb)

            # ---- mean_corr: mc[p, n] = keys[n*128+p, :] . t ----
            mc = small.tile([P, NT], fp32, tag="mc")
            for n in range(NT):
                kt = stream.tile([P, D], fp32, tag="ktile")
                nc.sync.dma_start(kt, k_dram[n * P:(n + 1) * P, :])
                tmp = stream.tile([P, D], fp32, tag="mctmp")
                nc.vector.tensor_mul(tmp, kt, t_b)
                nc.vector.reduce_sum(mc[:, n:n + 1], tmp, axis=AX)

            # flatten to [1, 2048] via DRAM bounce
            d_mc = dpool.tile([1, L], fp32, tag="d_mc")
            nc.sync.dma_start(d_mc.rearrange("o (n p) -> (o p) n", p=P), mc)
            mc_flat = small.tile([1, L], fp32, tag="mc_flat")
            nc.sync.dma_start(mc_flat, d_mc)

            # ---- top-8 ----
            mx8 = small.tile([1, 8], fp32, tag="mx8")
            mi8 = small.tile([1, 8], u32, tag="mi8")
            nc.vector.max(out=mx8, in_=mc_flat)
            nc.vector.max_index(out=mi8, in_max=mx8, in_values=mc_flat)

            # ---- softmax over top-4 ----
            e4 = small.tile([1, K_SEL], fp32, tag="e4")
            nc.scalar.activation(
                e4, mx8[0:1, 0:K_SEL], mybir.ActivationFunctionType.Exp,
                scale=float(SCALE),
            )
            s1 = small.tile([1, 1], fp32, tag="s1")
            nc.vector.reduce_sum(s1, e4, axis=AX)
            r1 = small.tile([1, 1], fp32, tag="r1")
            nc.vector.reciprocal(r1, s1)
            w4 = small.tile([1, K_SEL], fp32, tag="w4")
            nc.vector.tensor_scalar(w4, e4, r1[0:1, 0:1], None, op0=MUL)

            # broadcast w to [128, 4]
            ps_wb = ps_fe.tile([P, K_SEL], fp32, tag="fe")
            nc.tensor.matmul(ps_wb, ones_row, w4, start=True, stop=True)
            wb = small.tile([P, K_SEL], fp32, tag="wb_sb")
            nc.scalar.copy(wb, ps_wb)

            # ---- per-j scaled dynamic-slice copy (only 4 dynamic APs):
            #   ws[j] = w_j * vpT[:, (ct, s_j : s_j + 2048)] for both ct ----
            vp3 = vpT.rearrange("p (c x) -> p c x", c=2)  # [128, 2, 4096]
            # reuse the (now dead) vt slots for ws
            ws = [big.tile([P, 2, L], fp32, tag=f"vt{j}", name=f"ws{j}")
                  for j in range(K_SEL)]
            for j in range(K_SEL):
                eng = mybir.EngineType.Activation if j % 2 == 0 else mybir.EngineType.DVE
                s_j = nc.values_load(
                    mi8[0:1, j:j + 1].bitcast(i32),
                    engines=(eng,),
                    min_val=0, max_val=L - 1,
                    skip_runtime_bounds_check=True,
                )
                dyn = vp3[:, :, bass.ds(s_j, L)]
                if j % 2 == 0:
                    nc.scalar.activation(
                        ws[j], dyn, mybir.ActivationFunctionType.Copy,
                        scale=wb[:, j:j + 1])
                else:
                    nc.vector.tensor_scalar(
                        ws[j], dyn, wb[:, j:j + 1], None, op0=MUL)

            # Aw^T accumulate: ws0 += ws1; ws2 += ws3; ws0 += ws2
            nc.vector.tensor_add(ws[0], ws[0], ws[1])
            nc.vector.tensor_add(ws[2], ws[2], ws[3])
            nc.vector.tensor_add(ws[0], ws[0], ws[2])

            # ---- transpose AwT -> Aw tiles [128 l, 256 c] per (r, lp) ----
            aw = [[small.tile([P, CH], f32r, tag=f"aw{r}_{lp}", name=f"aw{r}_{lp}") for lp in range(4)]
                  for r in range(4)]
            for r in range(4):
                for ct in range(2):
                    for lp in range(4):
                        pt = ps_tp.tile([P, P], fp32, tag="tp")
                        nc.tensor.transpose(
                            pt, ws[0][:, ct, r * 512 + lp * P: r * 512 + (lp + 1) * P],
                            ident)
                        nc.scalar.copy(aw[r][lp][:, ct * P:(ct + 1) * P], pt)

            # ---- out rows: for r, cm: psum = sum_lp aw[r][lp][:,cm]  @ wo[lp] ----
            for r in range(4):
                for cm in range(2):
                    po = ps_mm.tile([P, D], fp32, tag="mm")
                    for lp in range(4):
                        nc.tensor.matmul(
                            po,
                            aw[r][lp][:, cm * P:(cm + 1) * P],
                            wo_sb[lp],
                            start=(lp == 0), stop=(lp == DK - 1),
                        )
                    ot = stream.tile([P, D], fp32, tag="otile")
                    nc.scalar.copy(ot, po)
                    row0 = r * 256 + cm * P
                    nc.sync.dma_start(out_dram[row0:row0 + P, :], ot)

    nc.compile()
    return nc


_NC_CACHE = None


def _get_nc():
    global _NC_CACHE
    if _NC_CACHE is None:
        _NC_CACHE = _build_nc()
    return _NC_CACHE


def _half_cols(half):
    d0 = 32 * half
    return np.array([(cl // 32) * 64 + d0 + cl % 32 for cl in range(CH)])


def _row_index(half):
    # device row r*256 + cl  ->  full-output row i
    d0 = 32 * half
    idx = np.empty(1024, np.int64)
    for r in range(4):
        for cl in range(CH):
            i = (d0 + cl % 32) * 32 + (cl // 32) * 4 + r
            idx[r * CH + cl] = i
    return idx


def make_in_maps(queries, keys, values, wq, wk, wv, wo):
    ones = np.ones((P, 1), np.float32)
    in_maps = []
    for c in range(N_CORES):
        b, half = c // 2, c % 2
        vt = np.ascontiguousarray(values[b].T)  # [512, 2048]
        wvh = np.ascontiguousarray(wv[:, _half_cols(half)])
        in_maps.append({
            "q": np.ascontiguousarray(queries[b]),
            "k": np.ascontiguousarray(keys[b]),
            "vt": vt,
            "wq": wq, "wk": wk, "wvh": wvh, "wo": wo,
            "ones": ones, "onesr": np.ones((1, P), np.float32),
        })
    return in_maps


def kernel(queries, keys, values, wq, wk, wv, wo, trace=False):
    import sys
    if "/opt/trn_rl_repo" not in sys.path:
        sys.path.insert(0, "/opt/trn_rl_repo")
    from concourse import bass_utils

    nc = _get_nc()
    in_maps = make_in_maps(queries, keys, values, wq, wk, wv, wo)
    res = bass_utils.run_bass_kernel_spmd(
        nc, in_maps, core_ids=list(range(N_CORES)), trace=trace,
    )
    out = np.empty((B, L, D), np.float32)
    for c in range(N_CORES):
        b, half = c // 2, c % 2
        out[b, _row_index(half), :] = res.results[c]["out"]
    if trace:
        return out, res
    return out
